# revision 63
# baseline (speedup 1.0000x reference)
"""Trainium2 Bass kernel for nn_BiSpikeNet: spiking net with per-step abs-mean
normalization + tiny MLP attention over time.

Strategy (8 NeuronCores, data-parallel over batch, 2 samples/core):
  Work in K-units K = m/c with c = d*v_th, x pre-scaled to x2 = x/c in fp16
  on the host. The recurrence per step t (per sample chain b):
    K_t   = x2_t + gamma_{t-1}*K_{t-1} - S_{t-1}        S in {0,1}
    Sigma = sum|K_t|  (over the [128 x 2048] slab)
    theta = k1*Sigma + e1;  gamma_t = 1/(k2*Sigma + e2)
    S_t   = (K_t >= theta)
  All slab tensors are fp16. Engine mapping per slab (constrained to the
  ops the walrus BIR verifier accepts: no gpsimd scalar_tensor_tensor or
  PSUM access, no DVE abs_max tensor_scalar):
    DVE:   gk = K*gamma (tensor_scalar 4x with [P,1] ptr), compare via
           tensor_scalar 4x (+row counts), column shares of the two TT
           joins, tiny theta/gamma ops (incl. reciprocal)
    Pool:  column shares of xs = x2 - S_prev and K = gk + xs (TT)
    Act:   relu(K) + row sums; Sigma|K| = 2*Sum relu(K) - Sum K where
           Sum K = Sum gk (free accum) + Sum x2 (host) - count(S_prev)
    PE:    Sigma combined+broadcast to [128,1] via signed ones matmuls
  Spikes stay resident in SBUF (fp16, 64KB/partition). The tiny attention
  MLP runs on-device; out[b] = sum_t aw[t]*S[t] via fp16 identity-scaled
  matmuls accumulating in PSUM plus DVE/Pool accumulation chunks for the
  tail, written to DRAM as fp16 and upcast on the host.
"""

import os
import numpy as np

P = 128
FREE = 2048
T = 8
BL = 2            # samples per core
NCORES = 8
NSLAB = T * BL
F = 256 * 32 * 32  # 262144
NH, HID = 4, 64

# --- tunable engine splits (sim-tuned) ---
XPB = 6            # x-slab DMA lookahead buffers
SCB = 2            # relu scratch buffers
GKB = 2            # gk buffers
ABS_DVE = 0        # relu(K) columns [FREE-ABS_DVE:FREE) on DVE, rest on Act
TT_POOL = 1024     # K=gk+xs columns [FREE-TT_POOL:FREE) on Pool, rest DVE
XS_POOL = 1024     # xs=x2-S columns [FREE-XS_POOL:FREE) on Pool, rest DVE
HF = 1024          # TT pipelining granule

_cache = {}
LAST_RESULT = None  # set to the BassKernelResults of the most recent run


def _build(vth, c, d, mode=3, repeat=1):
    import concourse.bacc as bacc
    import concourse.mybir as mybir
    import concourse.tile as tile

    dt = mybir.dt
    Alu = mybir.AluOpType
    Act = mybir.ActivationFunctionType

    nc = bacc.Bacc("TRN2", target_bir_lowering=False, debug=False,
                   num_devices=NCORES)

    x_d = nc.declare_dram_parameter("x", [NSLAB, P, FREE], dt.float16, isOutput=False)
    w1_d = nc.declare_dram_parameter("w1", [2, P, T], dt.float32, isOutput=False)
    b1_d = nc.declare_dram_parameter("b1", [2, P, 1], dt.float32, isOutput=False)
    w2_d = nc.declare_dram_parameter("w2", [2, P, T], dt.float32, isOutput=False)
    b2_d = nc.declare_dram_parameter("b2", [2, 2, T], dt.float32, isOutput=False)
    aw_d = nc.declare_dram_parameter("attw", [2, 2, 1], dt.float32, isOutput=False)
    gones_d = nc.declare_dram_parameter("gones", [P, 2], dt.float32, isOutput=False)
    x2row_d = nc.declare_dram_parameter("x2row", [P, NSLAB], dt.float32, isOutput=False)
    ident_d = nc.declare_dram_parameter("ident", [P, P], dt.float16, isOutput=False)
    out_d = nc.declare_dram_parameter("out", [BL, P, FREE], dt.float16, isOutput=True)
    dbg_d = None
    if mode < 3:
        dbg_d = nc.declare_dram_parameter("dbg", [1, 64], dt.float32, isOutput=True)

    k1 = float(np.float32(vth) / np.float32(F))                 # theta = k1*Sig + e1
    e1 = float(np.float32(1e-6) * np.float32(vth) / np.float32(c))
    k2 = float(np.float32(c) / (np.float32(d) * np.float32(F)))  # 1/gamma = k2*Sig + e2
    e2 = float(np.float32(1e-6) / np.float32(d))

    with tile.TileContext(nc) as tc:
        with (
            tc.tile_pool(name="xp", bufs=XPB) as xp,
            tc.tile_pool(name="scrp", bufs=SCB) as scrp,
            tc.tile_pool(name="gkp", bufs=GKB) as gkp,
            tc.tile_pool(name="persist", bufs=1) as pp,
            tc.tile_pool(name="small", bufs=4) as sp,
            tc.tile_pool(name="psmall", bufs=2, space="PSUM") as psm,
            tc.tile_pool(name="psb", bufs=2, space="PSUM") as psb,
            tc.tile_pool(name="psout", bufs=3, space="PSUM") as pso,
        ):
            # ---- aux / persistent tiles ----
            # x slabs 0/1 first on the sync queue so compute can start early;
            # the small weight DMAs ride the Activation engine's queue.
            x01 = []
            for i in range(2):
                xt0 = pp.tile([P, FREE], dt.float16, tag=f"x0{i}", name=f"xt0{i}")
                if i == 0:
                    # halves so t=0 relu can begin earlier
                    MID = FREE // 2
                    nc.sync.dma_start(xt0[:, 0:MID], x_d[i, :, 0:MID])
                    nc.sync.dma_start(xt0[:, MID:FREE], x_d[i, :, MID:FREE])
                else:
                    nc.sync.dma_start(xt0[:], x_d[i, :, :])
                x01.append(xt0)

            ones128 = pp.tile([P, 1], dt.float32, tag="ones128")
            nc.vector.memset(ones128[:], 1.0)
            ones_row = pp.tile([1, P], dt.float32, tag="ones_row")
            nc.vector.memset(ones_row[:], 1.0)
            onesq = pp.tile([P, P], dt.float32, tag="onesq")
            nc.gpsimd.memset(onesq[:], 1.0)
            twosq = pp.tile([P, P], dt.float32, tag="twosq")
            nc.gpsimd.memset(twosq[:], 2.0)
            negq = pp.tile([P, P], dt.float32, tag="negq")
            nc.gpsimd.memset(negq[:], -1.0)
            x2r = pp.tile([P, NSLAB], dt.float32, tag="x2r")
            nc.gpsimd.dma_start(x2r[:], x2row_d[:, :])
            identh = pp.tile([P, P], dt.float16, tag="identh")
            nc.gpsimd.dma_start(identh[:], ident_d[:, :])
            gones = pp.tile([P, 2], dt.float32, tag="gones")
            nc.gpsimd.dma_start(gones[:], gones_d[:, :])
            w1sb = []
            b1sb = []
            w2sb = []
            b2sb = []
            awsb = []
            for l in range(2):
                w1t = pp.tile([P, T], dt.float32, tag=f"w1_{l}")
                nc.gpsimd.dma_start(w1t[:], w1_d[l, :, :])
                w1sb.append(w1t)
                b1t = pp.tile([P, 1], dt.float32, tag=f"b1_{l}")
                nc.gpsimd.dma_start(b1t[:], b1_d[l, :, :])
                b1sb.append(b1t)
                w2t = pp.tile([P, T], dt.float32, tag=f"w2_{l}")
                nc.gpsimd.dma_start(w2t[:], w2_d[l, :, :])
                w2sb.append(w2t)
                b2t = pp.tile([2, T], dt.float32, tag=f"b2_{l}")
                nc.gpsimd.dma_start(b2t[:], b2_d[l, :, :])
                b2sb.append(b2t)
                awt = pp.tile([2, 1], dt.float32, tag=f"aw_{l}")
                nc.gpsimd.dma_start(awt[:], aw_d[l, :, :])
                awsb.append(awt)

            # preload the exp activation table during the DMA ramp
            warm = pp.tile([1, 1], dt.float32, tag="warm")
            nc.scalar.activation(warm[:], ones128[0:1, 0:1], Act.Exp)
            zcol = pp.tile([P, 1], dt.float32, tag="zcol")
            nc.vector.memset(zcol[:], 0.0)

            Kst = [pp.tile([P, FREE], dt.float16, tag=f"kst{b}", name=f"kst{b}")
                   for b in range(BL)]
            tmp = [pp.tile([P, FREE], dt.float16, tag=f"tmp{b}", name=f"tmp{b}")
                   for b in range(BL)]
            # theta (compare threshold) and gamma (state multiplier) as
            # [P,1] SBUF vectors per chain
            tht = [pp.tile([P, 1], dt.float32, tag=f"tht{b}", name=f"tht{b}")
                   for b in range(BL)]
            tgd = [pp.tile([P, 1], dt.float32, tag=f"tgd{b}", name=f"tgd{b}")
                   for b in range(BL)]
            spikes = [pp.tile([P, FREE], dt.float16, tag=f"s{i}", name=f"s{i}")
                      for i in range(NSLAB)]
            rowcnts = pp.tile([P, NSLAB], dt.float32, tag="rowcnts")

            for _rep in range(repeat):
                # ---- phase 1: the T-step recursion ----
                prevK = [None] * BL
                for t in range(T):
                    for b in range(BL):
                        i = t * BL + b
                        if _rep == 0 and t == 0:
                            xt = x01[b]
                        else:
                            xt = xp.tile([P, FREE], dt.float16, tag="xt")
                            nc.sync.dma_start(xt[:], x_d[i, :, :])
                        if t == 0:
                            K = xt
                        else:
                            # xs = x2 - S_prev: off the gamma critical path,
                            # split DVE / Pool in HF pieces
                            Sp = spikes[i - BL]
                            CX = FREE - XS_POOL
                            for lo in range(0, CX, HF):
                                hi = min(lo + HF, CX)
                                nc.vector.tensor_tensor(
                                    tmp[b][:, lo:hi], xt[:, lo:hi],
                                    Sp[:, lo:hi], Alu.subtract)
                            if XS_POOL:
                                nc.gpsimd.tensor_tensor(
                                    tmp[b][:, CX:FREE], xt[:, CX:FREE],
                                    Sp[:, CX:FREE], Alu.subtract)
                            # gk = K*gamma (DVE tensor_scalar 4x) + row sums
                            gk = gkp.tile([P, FREE], dt.float16, tag="gk")
                            rowgk = sp.tile([P, 1], dt.float32, tag="rowgk")
                            nc.vector.tensor_scalar(
                                gk[:], prevK[b][:], tgd[b][:, 0:1], 0.0,
                                Alu.mult, Alu.add, accum_out=rowgk[:])
                            # K = gk + xs: DVE + Pool column split, HF pieces
                            CT = FREE - TT_POOL
                            for lo in range(0, CT, HF):
                                hi = min(lo + HF, CT)
                                nc.vector.tensor_tensor(
                                    Kst[b][:, lo:hi], gk[:, lo:hi],
                                    tmp[b][:, lo:hi], Alu.add)
                            if TT_POOL:
                                nc.gpsimd.tensor_tensor(
                                    Kst[b][:, CT:FREE], gk[:, CT:FREE],
                                    tmp[b][:, CT:FREE], Alu.add)
                            K = Kst[b]

                        # Sigma|K| = 2*Sum relu(K) - Sum K, with
                        # Sum K = Sum gk + Sum x2 (host) - count(S_prev).
                        # Act computes relu+rowsum; PE combines the signed
                        # partials while broadcasting to all partitions.
                        scr = scrp.tile([P, FREE], dt.float16, tag="scr")
                        rowab = sp.tile([P, 2], dt.float32, tag="rowab")
                        ACT_A = FREE - ABS_DVE
                        nsrc = 0
                        if ACT_A > 0:
                            nc.scalar.activation(
                                scr[:, 0:ACT_A], K[:, 0:ACT_A], Act.Relu,
                                accum_out=rowab[:, nsrc:nsrc + 1])
                            nsrc += 1
                        if ACT_A < FREE:
                            nc.vector.tensor_scalar(
                                scr[:, ACT_A:FREE], K[:, ACT_A:FREE], 0.0, 0.0,
                                Alu.max, Alu.add,
                                accum_out=rowab[:, nsrc:nsrc + 1])
                            nsrc += 1

                        psS = psb.tile([P, 1], dt.float32, tag="psS")
                        nc.tensor.matmul(psS[:], negq[:], x2r[:, i:i + 1],
                                         start=True, stop=False)
                        if t > 0:
                            nc.tensor.matmul(psS[:], negq[:], rowgk[:],
                                             start=False, stop=False)
                            nc.tensor.matmul(psS[:], onesq[:],
                                             rowcnts[:, i - BL:i - BL + 1],
                                             start=False, stop=False)
                        for h in range(nsrc):
                            nc.tensor.matmul(psS[:], twosq[:], rowab[:, h:h + 1],
                                             start=False, stop=(h == nsrc - 1))
                        # gamma first (gates next step's gk), then theta
                        if t + 1 < T:
                            tb = sp.tile([P, 1], dt.float32, tag="tb")
                            nc.vector.tensor_scalar(tb[:], psS[:], k2, e2,
                                                    Alu.mult, Alu.add)
                            nc.vector.reciprocal(tgd[b][:, 0:1], tb[:])
                        nc.vector.tensor_scalar(tht[b][:, 0:1], psS[:], k1, e1,
                                                Alu.mult, Alu.add)

                        # compare: S = (K >= theta), fp16 {0,1}, + row counts
                        nc.vector.tensor_scalar(
                            spikes[i][:], K[:], tht[b][:, 0:1], 0.0,
                            Alu.is_ge, Alu.add, accum_out=rowcnts[:, i:i + 1])
                        prevK[b] = K

                if mode == 1:
                    for b in range(BL):
                        nc.sync.dma_start(out_d[b, :, 0:FREE], Kst[b][:])
                # ---- MLP attention over time (tiny), fully per-sample ----
                krows = []
                for b in (range(BL) if mode >= 2 else []):
                    # spike counts broadcast to all partitions in ONE matmul:
                    # psT[p,t] = sum_k onesq[k,p] * rowcnts[k, t*BL+b]
                    psT = psb.tile([P, T], dt.float32, tag="psS", name=f"psT{b}")
                    nc.tensor.matmul(psT[:], onesq[:], rowcnts[:, b::BL],
                                     start=True, stop=True)
                    if mode == 1:
                        summ_bc = sp.tile([P, T], dt.float32, tag=f"summbc{b}")
                        nc.scalar.copy(summ_bc[:], psT[:])
                        nc.sync.dma_start(dbg_d[0:1, b * T:(b + 1) * T],
                                          summ_bc[0:1, :])

                    mws = []
                    for l in range(2):
                        junk = sp.tile([P, T], dt.float32, tag=f"junk{l}{b}")
                        hraw = sp.tile([P, 1], dt.float32, tag=f"hraw{l}{b}")
                        nc.vector.scalar_tensor_tensor(
                            junk[:], w1sb[l][:], 1.0, psT[:],
                            Alu.mult, Alu.mult, accum_out=hraw[:])
                        # hcol = max(hraw + b1, 0) in one fused op
                        hcol = sp.tile([P, 1], dt.float32, tag=f"hcol{l}{b}")
                        nc.vector.scalar_tensor_tensor(
                            hcol[:], hraw[:], b1sb[l][:, 0:1], zcol[:],
                            Alu.add, Alu.max)
                        mc = sp.tile([P, T], dt.float32, tag=f"mc{l}{b}")
                        nc.vector.tensor_scalar(mc[:], w2sb[l][:], hcol[:, 0:1], None,
                                                Alu.mult)
                        psM = psm.tile([2, T], dt.float32, tag="psD", name="psM")
                        nc.tensor.matmul(psM[:], gones[:], mc[:], start=True, stop=True)
                        # mw = psM*aw + b2*aw  (b2*aw folded host-side into b2sb)
                        mw = sp.tile([2, T], dt.float32, tag=f"mw{l}{b}")
                        nc.vector.scalar_tensor_tensor(
                            mw[:], psM[:], awsb[l][:, 0:1], b2sb[l][:],
                            Alu.mult, Alu.add)
                        mws.append(mw)
                    psW = psm.tile([1, T], dt.float32, tag="psD", name="psW")
                    nc.tensor.matmul(psW[:], ones128[0:2, 0:1], mws[0][:],
                                     start=True, stop=False)
                    nc.tensor.matmul(psW[:], ones128[0:2, 0:1], mws[1][:],
                                     start=False, stop=True)
                    # weighted-map values are O(0.3) here, so the softmax is
                    # safe without max-subtraction
                    ex = sp.tile([1, T], dt.float32, tag=f"ex{b}")
                    nc.scalar.activation(ex[:], psW[:], Act.Exp)
                    zs = sp.tile([1, 1], dt.float32, tag=f"zs{b}")
                    nc.vector.tensor_reduce(zs[:], ex[:], mybir.AxisListType.X, Alu.add)
                    rz = sp.tile([1, 1], dt.float32, tag=f"rz{b}")
                    nc.vector.reciprocal(rz[:], zs[:])
                    krow_b = sp.tile([1, T], dt.float32, tag=f"krow{b}")
                    nc.vector.tensor_scalar(krow_b[:], ex[:], rz[0:1, 0:1], None,
                                            Alu.mult)
                    krows.append(krow_b)
                    if mode == 2:
                        nc.sync.dma_start(dbg_d[0:1, b * T:(b + 1) * T], krow_b[:])

                # ---- phase 2: out[b] = sum_t aw[t,b] * S[t,b] ----
                if mode >= 3:
                    NCH = FREE // 512
                    kI = {}
                    psKs = []
                    for b in range(BL):
                        psK = psb.tile([P, T], dt.float32, tag="psS", name=f"psK{b}")
                        nc.tensor.matmul(psK[:], ones_row[:], krows[b][:],
                                         start=True, stop=True)
                        psKs.append(psK)

                    def build_kI(b, eng):
                        for t in range(T):
                            kt = pp.tile([P, P], dt.float16, tag=f"ki{t}_{b}",
                                         name=f"ki{t}_{b}")
                            eng.tensor_scalar(kt[:], identh[:],
                                              psKs[b][:, t:t + 1],
                                              None, Alu.mult)
                            kI[(t, b)] = kt

                    def pe_job(b, ch):
                        lo = ch * 512
                        po = pso.tile([P, 512], dt.float32, tag="po")
                        for t in range(T):
                            i = t * BL + b
                            nc.tensor.matmul(po[:], kI[(t, b)][:],
                                             spikes[i][:, lo:lo + 512],
                                             start=(t == 0), stop=(t == T - 1))
                        posb = scrp.tile([P, 512], dt.float16, tag="posb",
                                         bufs=4, name="posb")
                        nc.scalar.copy(posb[:], po[:])
                        nc.sync.dma_start(out_d[b, :, lo:lo + 512], posb[:])

                    def acc_job(b, ch, eng, kp):
                        lo = ch * 512
                        e = nc.vector if eng == "dve" else nc.gpsimd
                        if eng == "dve":
                            acc = pp.tile([P, 512], dt.float16, tag=f"acc{eng}{ch}",
                                          name=f"acc{eng}{ch}")
                            e.tensor_scalar(acc[:], spikes[b][:, lo:lo + 512],
                                            kp[:, 0:1], None, Alu.mult)
                            for t in range(1, T):
                                i = t * BL + b
                                e.scalar_tensor_tensor(
                                    acc[:], spikes[i][:, lo:lo + 512],
                                    kp[:, t:t + 1], acc[:], Alu.mult, Alu.add)
                            nc.sync.dma_start(out_d[b, :, lo:lo + 512], acc[:])
                        else:
                            # gpsimd has no scalar_tensor_tensor: TS product +
                            # TT accumulate, ping-pong buffers
                            pr = pp.tile([P, 512], dt.float16, tag=f"pr{ch}",
                                         name=f"pr{ch}")
                            accA = pp.tile([P, 512], dt.float16, tag=f"accA{ch}",
                                           name=f"accA{ch}")
                            accB = pp.tile([P, 512], dt.float16, tag=f"accB{ch}",
                                           name=f"accB{ch}")
                            e.tensor_scalar(accA[:], spikes[b][:, lo:lo + 512],
                                            kp[:, 0:1], None, Alu.mult)
                            cur, nxt = accA, accB
                            for t in range(1, T):
                                i = t * BL + b
                                e.tensor_scalar(pr[:], spikes[i][:, lo:lo + 512],
                                                kp[:, t:t + 1], None, Alu.mult)
                                e.tensor_tensor(nxt[:], cur[:], pr[:], Alu.add)
                                cur, nxt = nxt, cur
                            nc.sync.dma_start(out_d[b, :, lo:lo + 512], cur[:])

                    # private per-engine copies of the weight rows (GPSIMD
                    # cannot read PSUM, and cross-engine reads of one tile
                    # serialize in the tile framework)
                    kb0 = sp.tile([P, T], dt.float32, tag="kb0")
                    nc.scalar.copy(kb0[:], psKs[0][:])
                    kpD = sp.tile([P, T], dt.float32, tag="kpD")
                    nc.vector.tensor_copy(kpD[:], psKs[1][:])
                    kpP = sp.tile([P, T], dt.float32, tag="kpP")
                    nc.scalar.copy(kpP[:], psKs[1][:])
                    psKs[0] = kb0
                    # emission order tuned so each engine starts ASAP:
                    # Pool builds kI-b0 (it frees first), DVE takes a b1 chunk
                    # then builds kI-b1, chasing PE's last job.
                    build_kI(0, nc.gpsimd)
                    acc_job(1, NCH - 1, "dve", kpD)
                    acc_job(1, NCH - 2, "pool", kpP)
                    for ch in range(NCH):
                        pe_job(0, ch)
                    build_kI(1, nc.vector)
                    pe_job(1, 0)
                    pe_job(1, 1)

    nc.compile()
    return nc


def _nc_for_inputs(inputs, **bkw):
    decay_param = np.float32(np.asarray(inputs["decay_param"], dtype=np.float32))
    v_th = np.float32(np.asarray(inputs["v_th"], dtype=np.float32))
    d = np.float32(1.0) / (np.float32(1.0) + np.float32(np.exp(-np.float64(decay_param))))
    c = np.float32(d * v_th)
    return _build(float(v_th), float(c), float(d), **bkw)


def _make_in_maps(inputs):
    x = np.asarray(inputs["x"], dtype=np.float32)
    decay_param = np.float32(np.asarray(inputs["decay_param"], dtype=np.float32))
    v_th = np.float32(np.asarray(inputs["v_th"], dtype=np.float32))
    W1 = np.asarray(inputs["W1"], dtype=np.float32)
    b1 = np.asarray(inputs["b1"], dtype=np.float32)
    W2 = np.asarray(inputs["W2"], dtype=np.float32)
    b2 = np.asarray(inputs["b2"], dtype=np.float32)
    att_w = np.asarray(inputs["att_w"], dtype=np.float32)

    Tn, B, C, H, W = x.shape
    assert (Tn, B, C * H * W) == (T, BL * NCORES, F)

    d = np.float32(1.0) / (np.float32(1.0) + np.float32(np.exp(-np.float64(decay_param))))
    c = np.float32(d * v_th)
    x2 = (x * (np.float32(1.0) / c)).astype(np.float16)

    # host-side rearrangement of the tiny MLP weights
    # h[b,n,h'] = relu(sum_t count[t]*(W1[n,h',t]/F) + b1);  uses (n,h') on partitions
    w1c = (W1 / np.float32(F)).reshape(NH * HID, T).reshape(2, P, T)
    b1c = b1.reshape(NH * HID).reshape(2, P, 1)
    w2c = W2.transpose(0, 2, 1).reshape(NH * HID, T).reshape(2, P, T)
    b2c = (b2 * att_w[:, None]).reshape(2, 2, T)   # b2*aw folded
    awc = att_w.reshape(2, 2, 1)
    gones = np.zeros((P, 2), dtype=np.float32)
    gones[0:64, 0] = 1.0
    gones[64:128, 1] = 1.0
    ident = np.eye(P, dtype=np.float16)

    aux = {"w1": np.ascontiguousarray(w1c, np.float32),
           "b1": np.ascontiguousarray(b1c, np.float32),
           "w2": np.ascontiguousarray(w2c, np.float32),
           "b2": np.ascontiguousarray(b2c, np.float32),
           "attw": np.ascontiguousarray(awc, np.float32),
           "gones": gones, "ident": ident}

    in_maps = []
    for m in range(NCORES):
        xm = x2[:, m * BL:(m + 1) * BL].reshape(NSLAB, P, FREE)
        # per-row sums of the fp16-rounded x2, for Sigma|K| reconstruction
        x2row = xm.astype(np.float32).sum(axis=2).T  # [P, NSLAB]
        im = {"x": np.ascontiguousarray(xm),
              "x2row": np.ascontiguousarray(x2row, np.float32)}
        im.update(aux)
        in_maps.append(im)
    return in_maps


def kernel(**inputs):
    global LAST_RESULT
    from concourse.bass_utils import run_bass_kernel_spmd

    decay_param = np.float32(np.asarray(inputs["decay_param"], dtype=np.float32))
    v_th = np.float32(np.asarray(inputs["v_th"], dtype=np.float32))

    d = np.float32(1.0) / (np.float32(1.0) + np.float32(np.exp(-np.float64(decay_param))))
    c = np.float32(d * v_th)

    key = (float(v_th), float(c), float(d))
    nc = _cache.get(key)
    if nc is None:
        nc = _build(float(v_th), float(c), float(d))
        _cache[key] = nc

    in_maps = _make_in_maps(inputs)
    B = np.asarray(inputs["x"]).shape[1]

    trace = os.environ.get("BISPIKE_PROFILE", "") == "1"
    res = run_bass_kernel_spmd(nc, in_maps, list(range(NCORES)), trace=trace)
    LAST_RESULT = res

    out = np.empty((B, F), dtype=np.float32)
    for m in range(NCORES):
        out[m * BL:(m + 1) * BL] = res.results[m]["out"].reshape(BL, F).astype(np.float32)
    return out
